# revision 18
# baseline (speedup 1.0000x reference)
"""Causal self-attention (B=4, T=2048, C=1024, H=16) on 8 trn2 NeuronCores.

Sharding: data-parallel over batch (4) x tensor-parallel over heads (2 groups
of 8).  Core c handles batch c//2, head group c%2.  Each core computes
qkv projection for its heads, causal flash-style attention, and a partial
output projection (over its 512 rows of w_proj).  The host sums the two
TP partials per batch and adds the bias.

Device layout notes:
  - host feeds x^T (feature-major) so the contraction dim (C) lands on SBUF
    partitions for the QKV matmuls with no on-device transpose.
  - Q^T,K^T produced feature-on-partition ([64h+d -> (p,sub)]), V produced
    token-on-partition with a ones column on both sides, so P@V and the
    softmax denominator come from one matmul ([V|1] for even heads,
    [1|V] for odd heads -> denominator row lands at psum partition 64/0).
  - S^T tiles ([t2,t1]) are computed per (head-pair, q-chunk); softmax is
    exp-without-max (scores are ~N(0,1); max over 268M scores ~ 6.5, safe
    in fp32), masked additively only on the 128-wide diagonal slab; fully
    masked columns are simply never computed/streamed.
  - normalization: DVE reciprocal of the denominator row + GPSIMD
    partition_broadcast + DVE multiply into O^T.
  - output projection consumes O^T directly as lhsT (contraction = head dim
    on partitions); host pre-permutes w_proj rows to match the O^T layout.
"""

import sys

sys.path.insert(0, "/opt/trn_rl_repo")

import numpy as np

import concourse.bass as bass
import concourse.bacc as bacc
import concourse.mybir as mybir
import concourse.tile as tile
from concourse.bass_utils import run_bass_kernel_spmd

F32 = mybir.dt.float32
P = 128
B, T, C = 4, 2048, 1024
H, D = 16, 64
NCORES = 8
TP = 2               # head-parallel groups
HL = H // TP         # 8 heads per core
CW = HL * D          # 512 head-cols per core
KS = C // P          # 8 contraction subtiles
NT = T // P          # 16 token tiles
MASK_NEG = -30000.0
SCALE = float(1.0 / np.sqrt(D))

_CACHE = {}


def _build_module():
    nc = bacc.Bacc("TRN2", target_bir_lowering=False, debug=False,
                   num_devices=NCORES)
    xT = nc.dram_tensor("xT", (P, KS, T), F32, kind="ExternalInput").ap()
    wqk = nc.dram_tensor("wqk", (8, P, KS, P), F32, kind="ExternalInput").ap()
    wv = nc.dram_tensor("wv", (P, KS, CW), F32, kind="ExternalInput").ap()
    wp = nc.dram_tensor("wp", (P, 4, C), F32, kind="ExternalInput").ap()
    msk = nc.dram_tensor("msk", (P, P), F32, kind="ExternalInput").ap()
    y = nc.dram_tensor("y", (NT, P, C), F32, kind="ExternalOutput").ap()

    Exp = mybir.ActivationFunctionType.Exp
    Add = mybir.AluOpType.add

    with tile.TileContext(nc) as tc, \
         tc.tile_pool(name="per", bufs=1) as per, \
         tc.tile_pool(name="strm", bufs=2) as strm, \
         tc.tile_pool(name="pp", bufs=2) as pp, \
         tc.tile_pool(name="pss", bufs=2, space="PSUM") as pss, \
         tc.tile_pool(name="pso", bufs=4, space="PSUM") as pso:

        # K^T rows r=64h+d live at (partition r%128, subtile r//128)
        k_sb = per.tile([P, 4, T], F32)
        # V: [t2 partition, t-tile, 512 head-cols]
        v_sb = per.tile([P, NT, CW], F32)
        ones_sb = per.tile([P, 64], F32)
        oT_sb = per.tile([P, 4, T], F32)
        mask_sb = per.tile([P, P], F32)
        wv_sb = per.tile([P, KS, CW], F32, tag="wbig")

        nc.sync.dma_start(mask_sb, msk)
        nc.sync.dma_start(wv_sb, wv)
        nc.vector.memset(ones_sb, 1.0)

        for half in range(2):
            t0 = half * 1024
            xt = strm.tile([P, KS, 1024], F32, tag="xt", bufs=1)
            nc.sync.dma_start(xt, xT[:, :, t0:t0 + 1024])
            q_sb = strm.tile([P, 4, 1024], F32, tag="q", bufs=1)

            # ---- phase A: Q^T (mt 0-3) and K^T (mt 4-7) for this half ----
            for mt in (4, 0, 5, 1, 6, 2, 7, 3):
                w_t = strm.tile([P, KS, P], F32, tag="wqk")
                nc.sync.dma_start(w_t, wqk[mt])
                ps_a = pss.tile([P, 1024], F32, tag="s")
                for cc in range(2):
                    for ks in range(KS):
                        nc.tensor.matmul(
                            ps_a[:, cc * 512:(cc + 1) * 512],
                            lhsT=w_t[:, ks, :],
                            rhs=xt[:, ks, cc * 512:(cc + 1) * 512],
                            start=(ks == 0), stop=(ks == KS - 1))
                if mt < 4:
                    nc.scalar.copy(q_sb[:, mt, :], ps_a)
                else:
                    nc.scalar.copy(k_sb[:, mt - 4, t0:t0 + 1024], ps_a)

            # ---- phase B: V for this half's 8 token tiles ----
            for tt8 in range(8):
                tt = half * 8 + tt8
                ps_v = pso.tile([P, CW], F32, tag="o")
                for ks in range(KS):
                    nc.tensor.matmul(
                        ps_v,
                        lhsT=xt[:, ks, tt8 * 128:(tt8 + 1) * 128],
                        rhs=wv_sb[:, ks, :],
                        start=(ks == 0), stop=(ks == KS - 1))
                nc.vector.tensor_copy(out=v_sb[:, tt, :], in_=ps_v)

            # ---- phase C: attention for this half's two q-chunks ----
            for cc in range(2):
                c = half * 2 + cc
                ntile = 4 * c + 4
                for pr in range(4):
                    h0, h1 = 2 * pr, 2 * pr + 1
                    o0 = pso.tile([P, 512], F32, tag="o")
                    o1 = pso.tile([P, 512], F32, tag="o")
                    for tt in range(ntile):
                        i = tt - 4 * c  # diagonal index (>=0 on diagonal)
                        col0 = 128 * i if i >= 0 else 0
                        s_ps = pss.tile([P, 2, 512], F32, tag="s")
                        for hh, pb in ((0, 0), (1, 64)):
                            nc.tensor.matmul(
                                s_ps[:, hh, col0:512],
                                lhsT=k_sb[pb:pb + 64, pr, tt * 128:(tt + 1) * 128],
                                rhs=q_sb[pb:pb + 64, pr,
                                         cc * 512 + col0:cc * 512 + 512],
                                start=True, stop=True)
                        if i >= 0:
                            nc.vector.tensor_tensor(
                                out=s_ps[:, :, col0:col0 + 128],
                                in0=s_ps[:, :, col0:col0 + 128],
                                in1=mask_sb[:, None, :].to_broadcast((P, 2, P)),
                                op=Add)
                        p_t = pp.tile([P, 2, 512], F32, tag="p")
                        nc.scalar.activation(
                            p_t[:, :, col0:512], s_ps[:, :, col0:512],
                            Exp, scale=SCALE)
                        st, sp = (tt == 0), (tt == ntile - 1)
                        # O'^T for head + 64-row replicated denominator, in
                        # disjoint column groups of the same PSUM bank.
                        nc.tensor.matmul(
                            o0[0:64, col0:512],
                            lhsT=v_sb[:, tt, h0 * 64:(h0 + 1) * 64],
                            rhs=p_t[:, 0, col0:512], start=st, stop=sp,
                            tile_position=(0, 0), skip_group_check=True)
                        nc.tensor.matmul(
                            o0[64:128, col0:512],
                            lhsT=ones_sb,
                            rhs=p_t[:, 0, col0:512], start=st, stop=sp,
                            tile_position=(0, 64), skip_group_check=True)
                        nc.tensor.matmul(
                            o1[0:64, col0:512],
                            lhsT=ones_sb,
                            rhs=p_t[:, 1, col0:512], start=st, stop=sp,
                            tile_position=(0, 0), skip_group_check=True)
                        nc.tensor.matmul(
                            o1[64:128, col0:512],
                            lhsT=v_sb[:, tt, h1 * 64:(h1 + 1) * 64],
                            rhs=p_t[:, 1, col0:512], start=st, stop=sp,
                            tile_position=(0, 64), skip_group_check=True)
                    # normalize: O^T[h] = O'^T[h] * (1/denom[h]); the recip is
                    # computed on the replica rows, then partition-shifted via
                    # SBUF->SBUF DMA to align with the O' rows.
                    r0 = pp.tile([P, 512], F32, tag="r", bufs=4)
                    r0s = pp.tile([P, 512], F32, tag="r", bufs=4)
                    nc.vector.reciprocal(r0[64:128, :], o0[64:128, :])
                    nc.sync.dma_start(r0s[0:64, :], r0[64:128, :])
                    nc.vector.tensor_mul(
                        oT_sb[0:64, pr, c * 512:(c + 1) * 512],
                        o0[0:64, :], r0s[0:64, :])
                    r1 = pp.tile([P, 512], F32, tag="r", bufs=4)
                    r1s = pp.tile([P, 512], F32, tag="r", bufs=4)
                    nc.vector.reciprocal(r1[0:64, :], o1[0:64, :])
                    nc.sync.dma_start(r1s[64:128, :], r1[0:64, :])
                    nc.vector.tensor_mul(
                        oT_sb[64:128, pr, c * 512:(c + 1) * 512],
                        o1[64:128, :], r1s[64:128, :])

        # ---- phase D: partial output projection ----
        wp_sb = per.tile([P, 4, C], F32, tag="wbig")
        nc.sync.dma_start(wp_sb, wp)
        for mt in range(NT):
            ps_y = pss.tile([P, 1024], F32, tag="s")
            for jo in range(4):
                for nn in range(2):
                    nc.tensor.matmul(
                        ps_y[:, nn * 512:(nn + 1) * 512],
                        lhsT=oT_sb[:, jo, mt * 128:(mt + 1) * 128],
                        rhs=wp_sb[:, jo, nn * 512:(nn + 1) * 512],
                        start=(jo == 0), stop=(jo == 3))
            y_sb = pp.tile([P, C], F32, tag="p")
            nc.scalar.copy(y_sb, ps_y)
            nc.sync.dma_start(y[mt], y_sb)

    nc.compile()
    return nc


def get_module():
    if "nc" not in _CACHE:
        _CACHE["nc"] = _build_module()
    return _CACHE["nc"]


def _wp_perm():
    # O^T row layout: (partition p, subtile jo) <-> head h = 2*jo + (p>=64),
    # dim d = p % 64; w_proj row (within this core's 512) = 64*h + d.
    p = np.arange(P)[:, None]
    jo = np.arange(4)[None, :]
    h = 2 * jo + (p >= 64)
    return (64 * h + p % 64).reshape(-1)


def make_core_inputs(x, w_qkv, w_proj, core):
    b, g = core // TP, core % TP
    xt = np.ascontiguousarray(x[b].T)                    # [C, T]
    xt = np.ascontiguousarray(xt.reshape(KS, P, T).transpose(1, 0, 2))
    qcols = w_qkv[:, g * CW:(g + 1) * CW]
    kcols = w_qkv[:, C + g * CW:C + (g + 1) * CW]
    wqk = np.concatenate([qcols, kcols], axis=1)         # [C, 1024]
    wqk = np.ascontiguousarray(
        wqk.reshape(KS, P, 8, P).transpose(2, 1, 0, 3))  # [mt, p, ko, m]
    wv = w_qkv[:, 2 * C + g * CW:2 * C + (g + 1) * CW]
    wv = np.ascontiguousarray(wv.reshape(KS, P, CW).transpose(1, 0, 2))
    wp = np.ascontiguousarray(
        w_proj[g * CW:(g + 1) * CW, :][_wp_perm()].reshape(P, 4, C))
    mask = np.where(np.arange(P)[:, None] <= np.arange(P)[None, :],
                    np.float32(0.0), np.float32(MASK_NEG))
    return {"xT": xt, "wqk": wqk, "wv": wv, "wp": wp,
            "msk": np.ascontiguousarray(mask, np.float32)}


def _run(inputs, trace=False):
    x = np.asarray(inputs["x"], np.float32)
    w_qkv = np.asarray(inputs["w_qkv"], np.float32)
    w_proj = np.asarray(inputs["w_proj"], np.float32)
    b_proj = np.asarray(inputs["b_proj"], np.float32)
    nc = get_module()
    in_maps = [make_core_inputs(x, w_qkv, w_proj, core)
               for core in range(NCORES)]
    res = run_bass_kernel_spmd(nc, in_maps, core_ids=list(range(NCORES)),
                               trace=trace)
    outs = [np.asarray(r["y"], np.float32).reshape(T, C) for r in res.results]
    yfull = np.empty((B, T, C), np.float32)
    for b in range(B):
        yfull[b] = outs[TP * b] + outs[TP * b + 1] + b_proj[None, :]
    return yfull, res


def kernel(**inputs):
    y, _ = _run(inputs, trace=False)
    return y


# revision 21
# speedup vs baseline: 46.4423x; 46.4423x over previous
"""Causal self-attention (B=4, T=2048, C=1024, H=16) on 8 trn2 NeuronCores.

Sharding: data-parallel over batch (4) x tensor-parallel over heads (2 groups
of 8).  Core c handles batch c//2, head group c%2.  Each core computes
qkv projection for its heads, causal flash-style attention, and a partial
output projection (over its 512 rows of w_proj).  The host sums the two
TP partials per batch and adds the bias.

Device layout notes:
  - host feeds x^T (feature-major) so the contraction dim (C) lands on SBUF
    partitions for the QKV matmuls with no on-device transpose.
  - Q^T,K^T produced feature-on-partition ([64h+d -> (p,sub)]), V produced
    token-on-partition.  P@V runs as two col-tiled matmuls per head pair:
    the head's V block in one 64-partition column group and an all-ones
    block in the other, so the softmax denominator arrives replicated
    across 64 psum partitions at no extra stream cost (col groups can run
    concurrently on the PE).
  - S^T tiles ([t2,t1]) are computed per (head-pair, q-chunk) with the two
    heads row-tiled (K=64 each, array rows 0-63 / 64-127); softmax is
    exp-without-max (scores are ~N(0,1); max over 268M scores ~ 6.5, safe
    in fp32), masked additively only on the 128-wide diagonal slab; fully
    masked columns are simply never computed/streamed.
  - normalization: DVE reciprocal of the replicated denominator rows, a
    partition-shifting SBUF->SBUF DMA to align them with the O' rows, and
    a DVE multiply into O^T.
  - output projection consumes O^T directly as lhsT (contraction = head dim
    on partitions); host pre-permutes w_proj rows to match the O^T layout.
"""

import sys

sys.path.insert(0, "/opt/trn_rl_repo")

import numpy as np

import concourse.bass as bass
import concourse.bacc as bacc
import concourse.mybir as mybir
import concourse.tile as tile
from concourse.bass_utils import run_bass_kernel_spmd

F32 = mybir.dt.float32
P = 128
B, T, C = 4, 2048, 1024
H, D = 16, 64
NCORES = 8
TP = 2               # head-parallel groups
HL = H // TP         # 8 heads per core
CW = HL * D          # 512 head-cols per core
KS = C // P          # 8 contraction subtiles
NT = T // P          # 16 token tiles
MASK_NEG = -30000.0
SCALE = float(1.0 / np.sqrt(D))

_CACHE = {}


def _build_module():
    nc = bacc.Bacc("TRN2", target_bir_lowering=False, debug=False,
                   num_devices=NCORES)
    xT = nc.dram_tensor("xT", (P, KS, T), F32, kind="ExternalInput").ap()
    wqk = nc.dram_tensor("wqk", (8, P, KS, P), F32, kind="ExternalInput").ap()
    wv = nc.dram_tensor("wv", (P, KS, CW), F32, kind="ExternalInput").ap()
    wp = nc.dram_tensor("wp", (P, 4, C), F32, kind="ExternalInput").ap()
    msk = nc.dram_tensor("msk", (P, P), F32, kind="ExternalInput").ap()
    y = nc.dram_tensor("y", (NT, P, C), F32, kind="ExternalOutput").ap()

    Exp = mybir.ActivationFunctionType.Exp
    Add = mybir.AluOpType.add

    with tile.TileContext(nc) as tc, \
         tc.tile_pool(name="per", bufs=1) as per, \
         tc.tile_pool(name="strm", bufs=2) as strm, \
         tc.tile_pool(name="pp", bufs=2) as pp, \
         tc.tile_pool(name="pss", bufs=2, space="PSUM") as pss, \
         tc.tile_pool(name="pso", bufs=4, space="PSUM") as pso, \
         tc.tile_pool(name="dscr", bufs=4, space="DRAM") as dscr:

        # K^T rows r=64h+d live at (partition r%128, subtile r//128)
        k_sb = per.tile([P, 4, T], F32)
        # V: [t2 partition, t-tile, head, 65]; cols 0-63 = V, col 64 = ones
        v_sb = per.tile([P, NT, HL, 65], F32)
        oT_sb = per.tile([P, 4, T], F32)
        mask_sb = per.tile([P, P], F32)
        wv_sb = per.tile([P, KS, CW], F32, tag="wbig")

        nc.sync.dma_start(mask_sb, msk)
        nc.sync.dma_start(wv_sb, wv)
        nc.vector.memset(v_sb[:, :, :, 64:65], 1.0)

        for half in range(2):
            t0 = half * 1024
            xt = strm.tile([P, KS, 1024], F32, tag="xt", bufs=1)
            nc.sync.dma_start(xt, xT[:, :, t0:t0 + 1024])
            q_sb = strm.tile([P, 4, 1024], F32, tag="q", bufs=1)

            # ---- phase A: Q^T (mt 0-3) and K^T (mt 4-7) for this half ----
            for mt in (4, 0, 5, 1, 6, 2, 7, 3):
                w_t = strm.tile([P, KS, P], F32, tag="wqk")
                nc.sync.dma_start(w_t, wqk[mt])
                ps_a = pss.tile([P, 1024], F32, tag="s")
                for cc in range(2):
                    for ks in range(KS):
                        nc.tensor.matmul(
                            ps_a[:, cc * 512:(cc + 1) * 512],
                            lhsT=w_t[:, ks, :],
                            rhs=xt[:, ks, cc * 512:(cc + 1) * 512],
                            start=(ks == 0), stop=(ks == KS - 1))
                if mt < 4:
                    nc.vector.tensor_copy(out=q_sb[:, mt, :], in_=ps_a)
                else:
                    nc.vector.tensor_copy(out=k_sb[:, mt - 4, t0:t0 + 1024],
                                          in_=ps_a)

            # ---- phase B: V for this half's 8 token tiles ----
            for tt8 in range(8):
                tt = half * 8 + tt8
                ps_v = pso.tile([P, CW], F32, tag="o")
                for ks in range(KS):
                    nc.tensor.matmul(
                        ps_v,
                        lhsT=xt[:, ks, tt8 * 128:(tt8 + 1) * 128],
                        rhs=wv_sb[:, ks, :],
                        start=(ks == 0), stop=(ks == KS - 1))
                nc.vector.tensor_copy(out=v_sb[:, tt, :, 0:64],
                                      in_=ps_v.rearrange("p (h d) -> p h d", h=HL))

            # ---- phase C: attention for this half's two q-chunks ----
            for cc in range(2):
                c = half * 2 + cc
                ntile = 4 * c + 4
                for pr in range(4):
                    h0, h1 = 2 * pr, 2 * pr + 1
                    o0 = pso.tile([P, 512], F32, tag="o")
                    o1 = pso.tile([P, 512], F32, tag="o")
                    for tt in range(ntile):
                        i = tt - 4 * c  # diagonal index (>=0 on diagonal)
                        col0 = 128 * i if i >= 0 else 0
                        s_ps = pss.tile([P, 2, 512], F32, tag="s")
                        for hh, pb in ((0, 0), (1, 64)):
                            nc.tensor.matmul(
                                s_ps[:, hh, col0:512],
                                lhsT=k_sb[pb:pb + 64, pr, tt * 128:(tt + 1) * 128],
                                rhs=q_sb[pb:pb + 64, pr,
                                         cc * 512 + col0:cc * 512 + 512],
                                start=True, stop=True)
                        if i >= 0:
                            nc.vector.tensor_tensor(
                                out=s_ps[:, :, col0:col0 + 128],
                                in0=s_ps[:, :, col0:col0 + 128],
                                in1=mask_sb[:, None, :].to_broadcast((P, 2, P)),
                                op=Add)
                        p_t = pp.tile([P, 2, 512], F32, tag="p")
                        nc.scalar.activation(
                            p_t[:, :, col0:512], s_ps[:, :, col0:512],
                            Exp, scale=SCALE)
                        st, sp = (tt == 0), (tt == ntile - 1)
                        # [V|1] lhsT: O'^T rows 0-63 + denominator row 64.
                        nc.tensor.matmul(
                            o0[0:65, col0:512],
                            lhsT=v_sb[:, tt, h0, 0:65],
                            rhs=p_t[:, 0, col0:512], start=st, stop=sp,
                            skip_group_check=True)
                        nc.tensor.matmul(
                            o1[0:65, col0:512],
                            lhsT=v_sb[:, tt, h1, 0:65],
                            rhs=p_t[:, 1, col0:512], start=st, stop=sp,
                            skip_group_check=True)
                    # normalize: O^T[h] = O'^T[h] * (1/denom[h]).  The
                    # reciprocal of the denominator row is replicated to 64
                    # partitions by a DRAM-roundtrip broadcast DMA (stride-0
                    # DRAM source APs are legal; SBUF ones are not), then
                    # multiplied in.  Odd heads get a partition-shifting
                    # SBUF->SBUF DMA into the upper half of O^T.
                    cs = slice(c * 512, (c + 1) * 512)
                    r0 = pp.tile([P, 512], F32, tag="r", bufs=6)
                    b0 = pp.tile([P, 512], F32, tag="r", bufs=6)
                    scr0 = dscr.tile([1, 512], F32)
                    nc.vector.reciprocal(r0[64:65, :], o0[64:65, :])
                    nc.sync.dma_start(scr0, r0[64:65, :])
                    nc.sync.dma_start(b0[0:64, :],
                                      scr0.to_broadcast((64, 512)))
                    nc.vector.tensor_mul(oT_sb[0:64, pr, cs],
                                         o0[0:64, :], b0[0:64, :])
                    r1 = pp.tile([P, 512], F32, tag="r", bufs=6)
                    b1 = pp.tile([P, 512], F32, tag="r", bufs=6)
                    t1s = pp.tile([P, 512], F32, tag="r", bufs=6)
                    scr1 = dscr.tile([1, 512], F32)
                    nc.vector.reciprocal(r1[64:65, :], o1[64:65, :])
                    nc.sync.dma_start(scr1, r1[64:65, :])
                    nc.sync.dma_start(b1[0:64, :],
                                      scr1.to_broadcast((64, 512)))
                    nc.vector.tensor_mul(t1s[0:64, :], o1[0:64, :], b1[0:64, :])
                    nc.sync.dma_start(oT_sb[64:128, pr, cs], t1s[0:64, :])

        # ---- phase D: partial output projection ----
        wp_sb = per.tile([P, 4, C], F32, tag="wbig")
        nc.sync.dma_start(wp_sb, wp)
        for mt in range(NT):
            ps_y = pss.tile([P, 1024], F32, tag="s")
            for jo in range(4):
                for nn in range(2):
                    nc.tensor.matmul(
                        ps_y[:, nn * 512:(nn + 1) * 512],
                        lhsT=oT_sb[:, jo, mt * 128:(mt + 1) * 128],
                        rhs=wp_sb[:, jo, nn * 512:(nn + 1) * 512],
                        start=(jo == 0), stop=(jo == 3))
            y_sb = pp.tile([P, C], F32, tag="p")
            nc.scalar.copy(y_sb, ps_y)
            nc.sync.dma_start(y[mt], y_sb)

    nc.compile()
    return nc


def get_module():
    if "nc" not in _CACHE:
        _CACHE["nc"] = _build_module()
    return _CACHE["nc"]


def _wp_perm():
    # O^T row layout: (partition p, subtile jo) <-> head h = 2*jo + (p>=64),
    # dim d = p % 64; w_proj row (within this core's 512) = 64*h + d.
    p = np.arange(P)[:, None]
    jo = np.arange(4)[None, :]
    h = 2 * jo + (p >= 64)
    return (64 * h + p % 64).reshape(-1)


def make_core_inputs(x, w_qkv, w_proj, core):
    b, g = core // TP, core % TP
    xt = np.ascontiguousarray(x[b].T)                    # [C, T]
    xt = np.ascontiguousarray(xt.reshape(KS, P, T).transpose(1, 0, 2))
    qcols = w_qkv[:, g * CW:(g + 1) * CW]
    kcols = w_qkv[:, C + g * CW:C + (g + 1) * CW]
    wqk = np.concatenate([qcols, kcols], axis=1)         # [C, 1024]
    wqk = np.ascontiguousarray(
        wqk.reshape(KS, P, 8, P).transpose(2, 1, 0, 3))  # [mt, p, ko, m]
    wv = w_qkv[:, 2 * C + g * CW:2 * C + (g + 1) * CW]
    wv = np.ascontiguousarray(wv.reshape(KS, P, CW).transpose(1, 0, 2))
    wp = np.ascontiguousarray(
        w_proj[g * CW:(g + 1) * CW, :][_wp_perm()].reshape(P, 4, C))
    mask = np.where(np.arange(P)[:, None] <= np.arange(P)[None, :],
                    np.float32(0.0), np.float32(MASK_NEG))
    return {"xT": xt, "wqk": wqk, "wv": wv, "wp": wp,
            "msk": np.ascontiguousarray(mask, np.float32)}


def _run(inputs, trace=False):
    x = np.asarray(inputs["x"], np.float32)
    w_qkv = np.asarray(inputs["w_qkv"], np.float32)
    w_proj = np.asarray(inputs["w_proj"], np.float32)
    b_proj = np.asarray(inputs["b_proj"], np.float32)
    nc = get_module()
    in_maps = [make_core_inputs(x, w_qkv, w_proj, core)
               for core in range(NCORES)]
    res = run_bass_kernel_spmd(nc, in_maps, core_ids=list(range(NCORES)),
                               trace=trace)
    outs = [np.asarray(r["y"], np.float32).reshape(T, C) for r in res.results]
    yfull = np.empty((B, T, C), np.float32)
    for b in range(B):
        yfull[b] = outs[TP * b] + outs[TP * b + 1] + b_proj[None, :]
    return yfull, res


def kernel(**inputs):
    y, _ = _run(inputs, trace=False)
    return y


# revision 23
# speedup vs baseline: 57.3204x; 1.2342x over previous
"""Causal self-attention (B=4, T=2048, C=1024, H=16) on 8 trn2 NeuronCores.

Sharding: data-parallel over batch (4) x tensor-parallel over heads (2 groups
of 8).  Core c handles batch c//2, head group c%2.  Each core computes
qkv projection for its heads, causal flash-style attention, and a partial
output projection (over its 512 rows of w_proj).  The host sums the two
TP partials per batch and adds the bias.

Device layout notes:
  - host feeds x^T (feature-major) so the contraction dim (C) lands on SBUF
    partitions for the QKV matmuls with no on-device transpose.
  - Q^T,K^T produced feature-on-partition ([64h+d -> (p,sub)]), V produced
    token-on-partition.  P@V runs as two col-tiled matmuls per head pair:
    the head's V block in one 64-partition column group and an all-ones
    block in the other, so the softmax denominator arrives replicated
    across 64 psum partitions at no extra stream cost (col groups can run
    concurrently on the PE).
  - S^T tiles ([t2,t1]) are computed per (head-pair, q-chunk) with the two
    heads row-tiled (K=64 each, array rows 0-63 / 64-127); softmax is
    exp-without-max (scores are ~N(0,1); max over 268M scores ~ 6.5, safe
    in fp32), masked additively only on the 128-wide diagonal slab; fully
    masked columns are simply never computed/streamed.
  - normalization: DVE reciprocal of the replicated denominator rows, a
    partition-shifting SBUF->SBUF DMA to align them with the O' rows, and
    a DVE multiply into O^T.
  - output projection consumes O^T directly as lhsT (contraction = head dim
    on partitions); host pre-permutes w_proj rows to match the O^T layout.
"""

import sys

sys.path.insert(0, "/opt/trn_rl_repo")

import numpy as np

import concourse.bass as bass
import concourse.bacc as bacc
import concourse.mybir as mybir
import concourse.tile as tile
from concourse.bass_utils import run_bass_kernel_spmd

F32 = mybir.dt.float32
P = 128
B, T, C = 4, 2048, 1024
H, D = 16, 64
NCORES = 8
TP = 2               # head-parallel groups
HL = H // TP         # 8 heads per core
CW = HL * D          # 512 head-cols per core
KS = C // P          # 8 contraction subtiles
NT = T // P          # 16 token tiles
MASK_NEG = -30000.0
SCALE = float(1.0 / np.sqrt(D))

_CACHE = {}


def _build_module():
    nc = bacc.Bacc("TRN2", target_bir_lowering=False, debug=False,
                   num_devices=NCORES)
    xT = nc.dram_tensor("xT", (P, KS, T), F32, kind="ExternalInput").ap()
    wqk = nc.dram_tensor("wqk", (8, P, KS, P), F32, kind="ExternalInput").ap()
    wv = nc.dram_tensor("wv", (P, KS, CW), F32, kind="ExternalInput").ap()
    wp = nc.dram_tensor("wp", (P, 4, C), F32, kind="ExternalInput").ap()
    msk = nc.dram_tensor("msk", (P, P), F32, kind="ExternalInput").ap()
    y = nc.dram_tensor("y", (NT, P, C), F32, kind="ExternalOutput").ap()

    Exp = mybir.ActivationFunctionType.Exp
    Add = mybir.AluOpType.add

    with tile.TileContext(nc) as tc, \
         tc.tile_pool(name="per", bufs=1) as per, \
         tc.tile_pool(name="strm", bufs=2) as strm, \
         tc.tile_pool(name="pp", bufs=2) as pp, \
         tc.tile_pool(name="pss", bufs=2, space="PSUM") as pss, \
         tc.tile_pool(name="pso", bufs=4, space="PSUM") as pso, \
         tc.tile_pool(name="dscr", bufs=4, space="DRAM") as dscr:

        # K^T rows r=64h+d live at (partition r%128, subtile r//128)
        k_sb = per.tile([P, 4, T], F32)
        # V: [t2 partition, t-tile, head, 65]; cols 0-63 = V, col 64 = ones
        v_sb = per.tile([P, NT, HL, 65], F32)
        oT_sb = per.tile([P, 4, T], F32)
        mask_sb = per.tile([P, P], F32)
        wv_sb = per.tile([P, KS, CW], F32, tag="wbig")

        nc.sync.dma_start(mask_sb, msk)
        nc.sync.dma_start(wv_sb, wv)
        nc.vector.memset(v_sb[:, :, :, 64:65], 1.0)

        for half in range(2):
            t0 = half * 1024
            xt = strm.tile([P, KS, 1024], F32, tag="xt", bufs=1)
            nc.sync.dma_start(xt, xT[:, :, t0:t0 + 1024])
            q_sb = strm.tile([P, 4, 1024], F32, tag="q", bufs=1)

            # ---- phase A: Q^T (mt 0-3) and K^T (mt 4-7) for this half ----
            for mt in (4, 0, 5, 1, 6, 2, 7, 3):
                w_t = strm.tile([P, KS, P], F32, tag="wqk")
                nc.sync.dma_start(w_t, wqk[mt])
                ps_a = pss.tile([P, 1024], F32, tag="s")
                for cc in range(2):
                    for ks in range(KS):
                        nc.tensor.matmul(
                            ps_a[:, cc * 512:(cc + 1) * 512],
                            lhsT=w_t[:, ks, :],
                            rhs=xt[:, ks, cc * 512:(cc + 1) * 512],
                            start=(ks == 0), stop=(ks == KS - 1))
                if mt < 4:
                    nc.vector.tensor_copy(out=q_sb[:, mt, :], in_=ps_a)
                else:
                    nc.vector.tensor_copy(out=k_sb[:, mt - 4, t0:t0 + 1024],
                                          in_=ps_a)

            # ---- phase B: V for this half's 8 token tiles ----
            for tt8 in range(8):
                tt = half * 8 + tt8
                ps_v = pso.tile([P, CW], F32, tag="o")
                for ks in range(KS):
                    nc.tensor.matmul(
                        ps_v,
                        lhsT=xt[:, ks, tt8 * 128:(tt8 + 1) * 128],
                        rhs=wv_sb[:, ks, :],
                        start=(ks == 0), stop=(ks == KS - 1))
                nc.vector.tensor_copy(out=v_sb[:, tt, :, 0:64],
                                      in_=ps_v.rearrange("p (h d) -> p h d", h=HL))

            # ---- phase C: attention for this half's two q-chunks ----
            for cc in range(2):
                c = half * 2 + cc
                ntile = 4 * c + 4
                for pr in range(4):
                    h0, h1 = 2 * pr, 2 * pr + 1
                    o0 = pso.tile([P, 512], F32, tag="o")
                    o1 = pso.tile([P, 512], F32, tag="o")
                    for tt in range(ntile):
                        i = tt - 4 * c  # diagonal index (>=0 on diagonal)
                        col0 = 128 * i if i >= 0 else 0
                        s_ps = pss.tile([P, 2, 512], F32, tag="s")
                        for hh, pb in ((0, 0), (1, 64)):
                            nc.tensor.matmul(
                                s_ps[:, hh, col0:512],
                                lhsT=k_sb[pb:pb + 64, pr, tt * 128:(tt + 1) * 128],
                                rhs=q_sb[pb:pb + 64, pr,
                                         cc * 512 + col0:cc * 512 + 512],
                                start=True, stop=True)
                        if i >= 0:
                            nc.vector.tensor_tensor(
                                out=s_ps[:, :, col0:col0 + 128],
                                in0=s_ps[:, :, col0:col0 + 128],
                                in1=mask_sb[:, None, :].to_broadcast((P, 2, P)),
                                op=Add)
                        p_t = pp.tile([P, 2, 512], F32, tag="p")
                        nc.scalar.activation(
                            p_t[:, :, col0:512], s_ps[:, :, col0:512],
                            Exp, scale=SCALE)
                        st, sp = (tt == 0), (tt == ntile - 1)
                        # [V|1] lhsT: O'^T rows 0-63 + denominator row 64.
                        nc.tensor.matmul(
                            o0[0:65, col0:512],
                            lhsT=v_sb[:, tt, h0, 0:65],
                            rhs=p_t[:, 0, col0:512], start=st, stop=sp,
                            skip_group_check=True)
                        nc.tensor.matmul(
                            o1[0:65, col0:512],
                            lhsT=v_sb[:, tt, h1, 0:65],
                            rhs=p_t[:, 1, col0:512], start=st, stop=sp,
                            skip_group_check=True)
                    # normalize: O^T[h] = O'^T[h] * (1/denom[h]).  The
                    # reciprocal of the denominator row is replicated to 64
                    # partitions by a DRAM-roundtrip broadcast DMA (stride-0
                    # DRAM source APs are legal; SBUF ones are not), then
                    # multiplied in.  Odd heads get a partition-shifting
                    # SBUF->SBUF DMA into the upper half of O^T.
                    cs = slice(c * 512, (c + 1) * 512)
                    r0 = pp.tile([P, 512], F32, tag="r", bufs=6)
                    b0 = pp.tile([P, 512], F32, tag="r", bufs=6)
                    scr0 = dscr.tile([1, 512], F32)
                    nc.vector.reciprocal(r0[64:65, :], o0[64:65, :])
                    nc.sync.dma_start(scr0, r0[64:65, :])
                    nc.sync.dma_start(b0[0:64, :],
                                      scr0.to_broadcast((64, 512)))
                    nc.vector.tensor_mul(oT_sb[0:64, pr, cs],
                                         o0[0:64, :], b0[0:64, :])
                    r1 = pp.tile([P, 512], F32, tag="r", bufs=6)
                    b1 = pp.tile([P, 512], F32, tag="r", bufs=6)
                    t1s = pp.tile([P, 512], F32, tag="r", bufs=6)
                    scr1 = dscr.tile([1, 512], F32)
                    nc.vector.reciprocal(r1[64:65, :], o1[64:65, :])
                    nc.sync.dma_start(scr1, r1[64:65, :])
                    nc.sync.dma_start(b1[0:64, :],
                                      scr1.to_broadcast((64, 512)))
                    nc.vector.tensor_mul(t1s[0:64, :], o1[0:64, :], b1[0:64, :])
                    nc.sync.dma_start(oT_sb[64:128, pr, cs], t1s[0:64, :])

        # ---- phase D: partial output projection ----
        wp_sb = per.tile([P, 4, C], F32, tag="wbig")
        nc.sync.dma_start(wp_sb, wp)
        for mt in range(NT):
            ps_y = pss.tile([P, 1024], F32, tag="s")
            for jo in range(4):
                for nn in range(2):
                    nc.tensor.matmul(
                        ps_y[:, nn * 512:(nn + 1) * 512],
                        lhsT=oT_sb[:, jo, mt * 128:(mt + 1) * 128],
                        rhs=wp_sb[:, jo, nn * 512:(nn + 1) * 512],
                        start=(jo == 0), stop=(jo == 3))
            y_sb = pp.tile([P, C], F32, tag="p")
            nc.scalar.copy(y_sb, ps_y)
            nc.sync.dma_start(y[mt], y_sb)

    nc.compile()
    return nc


def get_module():
    if "nc" not in _CACHE:
        _CACHE["nc"] = _build_module()
    return _CACHE["nc"]


def _wp_perm():
    # O^T row layout: (partition p, subtile jo) <-> head h = 2*jo + (p>=64),
    # dim d = p % 64; w_proj row (within this core's 512) = 64*h + d.
    p = np.arange(P)[:, None]
    jo = np.arange(4)[None, :]
    h = 2 * jo + (p >= 64)
    return (64 * h + p % 64).reshape(-1)


def make_core_inputs(x, w_qkv, w_proj, core):
    b, g = core // TP, core % TP
    xt = np.ascontiguousarray(x[b].T)                    # [C, T]
    xt = np.ascontiguousarray(xt.reshape(KS, P, T).transpose(1, 0, 2))
    qcols = w_qkv[:, g * CW:(g + 1) * CW]
    kcols = w_qkv[:, C + g * CW:C + (g + 1) * CW]
    wqk = np.concatenate([qcols, kcols], axis=1)         # [C, 1024]
    wqk = np.ascontiguousarray(
        wqk.reshape(KS, P, 8, P).transpose(2, 1, 0, 3))  # [mt, p, ko, m]
    wv = w_qkv[:, 2 * C + g * CW:2 * C + (g + 1) * CW]
    wv = np.ascontiguousarray(wv.reshape(KS, P, CW).transpose(1, 0, 2))
    wp = np.ascontiguousarray(
        w_proj[g * CW:(g + 1) * CW, :][_wp_perm()].reshape(P, 4, C))
    mask = np.where(np.arange(P)[:, None] <= np.arange(P)[None, :],
                    np.float32(0.0), np.float32(MASK_NEG))
    return {"xT": xt, "wqk": wqk, "wv": wv, "wp": wp,
            "msk": np.ascontiguousarray(mask, np.float32)}


def _run(inputs, trace=False):
    x = np.asarray(inputs["x"], np.float32)
    w_qkv = np.asarray(inputs["w_qkv"], np.float32)
    w_proj = np.asarray(inputs["w_proj"], np.float32)
    b_proj = np.asarray(inputs["b_proj"], np.float32)
    nc = get_module()
    in_maps = [make_core_inputs(x, w_qkv, w_proj, core)
               for core in range(NCORES)]
    res = run_bass_kernel_spmd(nc, in_maps, core_ids=list(range(NCORES)),
                               trace=trace)
    outs = [np.asarray(r["y"], np.float32).reshape(T, C) for r in res.results]
    yfull = np.empty((B, T, C), np.float32)
    for b in range(B):
        yfull[b] = outs[TP * b] + outs[TP * b + 1] + b_proj[None, :]
    return yfull, res


def kernel(**inputs):
    y, _ = _run(inputs, trace=False)
    return y
